# revision 31
# baseline (speedup 1.0000x reference)
"""3-layer GCN (message passing) on 8 Trainium2 NeuronCores.

Strategy: partition destination nodes (and their incoming edges) across the
8 cores; replicate the 64x64 weights; the segment-sum runs locally per dst
partition against a full replicated source-feature table that is rebuilt and
all-gathered between layers.

Math: per layer  h' = LR(segsum(w * (h@W)[src])) = LR(segsum(w*h[src]) @ W),
so each layer gathers from a table T_l that already has the weight folded in:
  T_1 = x,  T_{l+1} = (LR-output of layer l) @ W_{l+1}.

v2 performance structure (vs v1):
  - Edges ordered (window, half, group): windows of G dst-groups; per window
    ONE big dma_gather call per table half (A/B int16 views). Big calls
    amortize the ~1us SWDGE fixed cost per gather (was 147 calls/layer of
    8 chunks; now ~26 calls of up to 45 chunks).
  - edge_w folded into the gathered features (64 cols on DVE) instead of the
    one-hot scatter matrix (128 cols); the one-hot S is pure 0/1.
  - PSUM: one accumulation bank per live dst-group; window size G bounds
    concurrent groups (PSUM has 8 banks: 5 agg + 1 transpose + 2 weights).
  - Every (group, half) segment padded to full 128-slot chunks so chunk
    boundaries never straddle groups (pad slots gather row 0 / halfA with
    w=0, masked by the w-fold).
"""

import os
import sys

sys.path.insert(0, "/opt/trn_rl_repo")

import numpy as np
import ml_dtypes

from concourse import bass, bacc, tile, mybir
from concourse.bass_utils import run_bass_kernel_spmd

NC = 8
D = 64
NEG_SLOPE = 0.01
GROUP = 128            # dst nodes per PSUM accumulation group
WIN = 2                # dst-groups per window (bounds live PSUM banks)
MAX_CALL_CHUNKS = 16   # chunks (of 128 idxs) per dma_gather call
DMA_SCRATCH = 65536    # SWDGE descriptor carveout bytes/partition
NQ = 4                 # SWDGE queues
HALFA_MAX = 32000      # src id threshold for table view A vs B

BF16 = mybir.dt.bfloat16
F32 = mybir.dt.float32
I16 = mybir.dt.int16
F8 = mybir.dt.float8e4

LAST_EXEC_NS = None    # set when KERNEL_PROFILE=1


def _install_profile_shim():
    """Provide antenv.axon_hooks (NTFF profiling) if the image lacks it."""
    import types

    if "antenv.axon_hooks" in sys.modules:
        return
    mod = types.ModuleType("antenv.axon_hooks")
    holder = [None]
    mod.set_axon_ntff_profile_hook = lambda h: holder.__setitem__(0, h)
    mod.get_axon_ntff_profile_hook = lambda: holder[0]
    sys.modules["antenv.axon_hooks"] = mod
    try:
        import antenv

        antenv.axon_hooks = mod
    except ImportError:
        pass
    try:
        from trn_agent_boot.trn_boot import _ntff_profile_via_ctypes

        h = _ntff_profile_via_ctypes("/opt/axon/libaxon_pjrt.so")
        if h is not None:
            mod.set_axon_ntff_profile_hook(h)
    except Exception:
        pass


def _prep_edges(src, dst, w, n_nodes, nodes_per_core):
    """Partition/sort/pad edges; build the static per-core windowed schedule.

    Edge order: window-major, then half (A, B), then group, then chunk.
    Every (group, half) segment is padded to k*128 slots.

    Returns (sched, per_core).
    sched: dict with n_groups, windows (list of dicts:
        groups, calls=[(half, c0, chunks=[(global_chunk, group)])]),
        total_chunks, first_chunk[g], last_chunk[g], offB_l1, offB_l23,
        rows_pad.
    per_core[c]: idx1, idx23 ([tc,128] int16), w, dl ([tc,128] f32/i16).
    """
    n_groups = int(np.ceil(nodes_per_core / GROUP))
    rows_pad = n_groups * GROUP
    offB_l1 = max(0, n_nodes - 32768)
    offB_l23 = max(0, NC * rows_pad - 32768)

    per_core_edges = []
    counts = np.zeros((NC, n_groups, 2), np.int64)
    for c in range(NC):
        lo, hi = c * nodes_per_core, (c + 1) * nodes_per_core
        m = (dst >= lo) & (dst < hi)
        es, ed, ew = src[m], dst[m] - lo, w[m]
        grp = ed // GROUP
        half = (es >= HALFA_MAX).astype(np.int64)
        order = np.lexsort((half, grp))
        es, ed, ew, grp, half = es[order], ed[order], ew[order], grp[order], half[order]
        per_core_edges.append((es, ed, ew, grp, half))
        for g in range(n_groups):
            gm = grp == g
            counts[c, g, 0] = int((gm & (half == 0)).sum())
            counts[c, g, 1] = int((gm & (half == 1)).sum())

    # static chunk counts: 128-padded max over cores; >= 1 chunk for half A
    mx = counts.max(axis=0)  # [n_groups, 2]
    k = (mx + GROUP - 1) // GROUP
    k[:, 0] = np.maximum(k[:, 0], 1)

    # windowed schedule: windows carry per-half chunk lists; the split into
    # dma_gather calls happens at build time (can differ per layer).
    windows = []
    first_chunk = np.full(n_groups, -1, np.int64)
    last_chunk = np.full(n_groups, -1, np.int64)
    ci = 0
    chunk_meta = []  # (group, half) per global chunk, in stream order
    for g0 in range(0, n_groups, WIN):
        groups = list(range(g0, min(g0 + WIN, n_groups)))
        halves = []
        for h in range(2):
            chunks = []
            for g in groups:
                for _ in range(int(k[g, h])):
                    chunks.append((ci, g))
                    chunk_meta.append((g, h))
                    if first_chunk[g] < 0:
                        first_chunk[g] = ci
                    last_chunk[g] = ci
                    ci += 1
            halves.append(chunks)
        windows.append(dict(groups=groups, halves=halves))
    total_chunks = ci

    # pack per-core arrays following the same stream order
    per_core = []
    for c in range(NC):
        es, ed, ew, grp, half = per_core_edges[c]
        idx1 = np.zeros((total_chunks, GROUP), np.int16)
        idx23 = np.zeros((total_chunks, GROUP), np.int16)
        wv = np.zeros((total_chunks, GROUP), np.float32)
        dl = np.zeros((total_chunks, GROUP), np.int16)
        # iterate (group, half) segments in the same order as chunk_meta
        seg_rows = {}
        for g in range(n_groups):
            for h in range(2):
                gm = (grp == g) & (half == h)
                seg_rows[(g, h)] = (es[gm], ed[gm], ew[gm])
        # assign chunk rows
        seg_pos = {}
        for cidx, (g, h) in enumerate(chunk_meta):
            pos = seg_pos.get((g, h), 0)
            s_, d_, w_ = seg_rows[(g, h)]
            n = len(s_)
            sl = slice(pos * GROUP, pos * GROUP + GROUP)
            buf_s = np.zeros(GROUP, np.int64)
            buf_d = np.zeros(GROUP, np.int64)
            buf_w = np.zeros(GROUP, np.float32)
            take = max(0, min(GROUP, n - pos * GROUP))
            if take:
                buf_s[:take] = s_[sl][:take]
                buf_d[:take] = d_[sl][:take] - g * GROUP
                buf_w[:take] = w_[sl][:take]
            if h == 1:
                buf_s[take:] = HALFA_MAX  # valid row for the B view
            i1 = np.where(buf_s < HALFA_MAX, buf_s, buf_s - offB_l1)
            s23 = (buf_s // nodes_per_core) * rows_pad + buf_s % nodes_per_core
            i23 = np.where(buf_s < HALFA_MAX, s23, s23 - offB_l23)
            assert i1.max() < 32768 and i1.min() >= 0
            assert i23.max() < 32768 and i23.min() >= 0
            idx1[cidx] = i1.astype(np.int16)
            idx23[cidx] = i23.astype(np.int16)
            wv[cidx] = buf_w
            dl[cidx] = buf_d.astype(np.int16)
            seg_pos[(g, h)] = pos + 1
        per_core.append(dict(idx1=idx1, idx23=idx23, w=wv, dl=dl))

    sched = dict(n_groups=n_groups, windows=windows, total_chunks=total_chunks,
                 first_chunk=first_chunk, last_chunk=last_chunk,
                 rows_pad=rows_pad, offB_l1=offB_l1, offB_l23=offB_l23)
    return sched, per_core


def _wrap_idx(idx_chunks):
    """[n_chunks,128] int16 -> SBUF wrap layout [128, n_chunks*8].

    dma_gather reads index at stream position p from (partition p%16,
    col p//16), replicated across the 8 q7 core groups (x8 on partitions).
    Calls slice contiguous column ranges, so pack per chunk: chunk i's 128
    positions occupy cols [8i, 8i+8).
    """
    n = idx_chunks.shape[0]
    w16 = idx_chunks.reshape(n * 8, 16).T  # [16, n*8]
    return np.tile(w16, (8, 1)).copy()


def _build_nc(n_nodes, sched):
    nodes_per_core = n_nodes // NC
    n_groups = sched["n_groups"]
    total_chunks = sched["total_chunks"]
    rows_pad = sched["rows_pad"]
    tab_rows = NC * rows_pad
    first_chunk = sched["first_chunk"]
    last_chunk = sched["last_chunk"]

    nc = bacc.Bacc("TRN2", target_bir_lowering=False, debug=False,
                   num_devices=NC, num_swdge_queues=NQ,
                   dynamic_dma_scratch_size=DMA_SCRATCH)

    # ---- I/O ----
    xdup_d = nc.dram_tensor("xdup", [n_nodes, 2 * D], BF16, kind="ExternalInput")
    idx1_d = nc.dram_tensor("idx1", [128, total_chunks * 8], I16, kind="ExternalInput")
    idx23_d = nc.dram_tensor("idx23", [128, total_chunks * 8], I16, kind="ExternalInput")
    wv_d = nc.dram_tensor("wv", [128, total_chunks], BF16, kind="ExternalInput")
    # one-hot scatter matrices, fp8 0/1, [128 edge-rows, total_chunks*128]
    s8_d = nc.dram_tensor("s8", [128, total_chunks * 128], F8, kind="ExternalInput")
    ident_d = nc.dram_tensor("ident", [128, 128], BF16, kind="ExternalInput")
    ws_d = nc.dram_tensor("ws", [3 * D, D], BF16, kind="ExternalInput")  # W1;W2;W3
    out_d = nc.dram_tensor("out", [nodes_per_core, D], F32, kind="ExternalOutput")

    # ---- tables ----
    g2_loc = nc.dram_tensor("g2_loc", [rows_pad, 2 * D], BF16, kind="Internal")
    g3_loc = nc.dram_tensor("g3_loc", [rows_pad, 2 * D], BF16, kind="Internal")
    g2_full = nc.dram_tensor("g2_full", [tab_rows, 2 * D], BF16, kind="Internal",
                             addr_space="Shared")
    g3_full = nc.dram_tensor("g3_full", [tab_rows, 2 * D], BF16, kind="Internal",
                             addr_space="Shared")

    offB_l1 = sched["offB_l1"]
    offB_l23 = sched["offB_l23"]

    with tile.TileContext(nc) as tc:
        with (
            tc.tile_pool(name="res", bufs=1) as res,
            tc.tile_pool(name="gp", bufs=8) as gpool,
            tc.tile_pool(name="wp", bufs=8) as wpool,
            tc.tile_pool(name="sp", bufs=8) as spool,
            tc.tile_pool(name="ep", bufs=4) as epool,
            tc.tile_pool(name="ps_agg", bufs=4, space="PSUM") as ps_agg,
            tc.tile_pool(name="ps_tr", bufs=1, space="PSUM") as ps_tr,
            tc.tile_pool(name="ps_w", bufs=1, space="PSUM") as ps_w,
            tc.tile_pool(name="ps_c", bufs=2, space="PSUM") as ps_c,
        ):
            idx1_t = res.tile([128, total_chunks * 8], I16)
            idx23_t = res.tile([128, total_chunks * 8], I16)
            wv_t = res.tile([128, total_chunks], BF16)
            ident_t = res.tile([128, 128], BF16)
            ws_t = res.tile([64, 3 * D], BF16)  # W_l at [:, 64l:64l+64]
            nc.sync.dma_start(idx1_t[:], idx1_d.ap())
            nc.sync.dma_start(idx23_t[:], idx23_d.ap())
            nc.sync.dma_start(wv_t[:], wv_d.ap())
            nc.sync.dma_start(ident_t[:], ident_d.ap())
            for l in range(3):
                nc.sync.dma_start(
                    ws_t[:, 64 * l : 64 * l + 64],
                    bass.AP(ws_d, l * 64 * 64, [[64, 64], [1, 64]]))

            qctr = [0]

            # pre-zero gather buffers: pad-slot gathers read real table rows,
            # but buffers must start finite (w=0 masks them in the fold).
            for _z in range(8):
                zt = gpool.tile([128, MAX_CALL_CHUNKS, 128], BF16, tag="g")
                nc.vector.memset(zt[:], 0.0)

            def table_view(t_d, nrows, offB):
                va = min(32768, nrows)
                apA = bass.AP(t_d, 0, [[2 * D, va], [1, 2 * D]])
                apB = bass.AP(t_d, offB * 2 * D, [[2 * D, va], [1, 2 * D]])
                return apA, apB

            def do_layer(table_aps, idx_t, emit_epilogue, maxck=MAX_CALL_CHUNKS):
                """Emit one layer's windows. emit_epilogue(g, agg_psum)."""
                open_aggs = {}
                for win in sched["windows"]:
                    calls = []
                    for h in range(2):
                        hchunks = win["halves"][h]
                        for s in range(0, len(hchunks), maxck):
                            part = hchunks[s : s + maxck]
                            if part:
                                calls.append((h, part[0][0], part))
                    for (h, c0, chunks) in calls:
                        nk = len(chunks)
                        ni = nk * 128
                        gt = gpool.tile([128, nk, 128], BF16, tag="g")
                        nc.gpsimd.dma_gather(
                            gt[:],
                            table_aps[h],
                            idx_t[:, c0 * 8 : c0 * 8 + ni // 16],
                            ni, ni, 128,
                            single_packet=False,
                            queue_num=qctr[0] % NQ,
                        )
                        qctr[0] += 1
                        # stage this call's w columns into PSUM via an
                        # identity matmul (PE is the only engine allowed to
                        # write PSUM). The fold then reads its second operand
                        # from PSUM, so DVE never takes the shared SBUF port
                        # that would lock GpSimd out of SWDGE descriptor
                        # generation for the whole instruction.
                        w_c = ps_c.tile([128, 1, nk], F32, tag="wc")
                        nc.tensor.matmul(w_c[:], ident_t[:],
                                         wv_t[:, c0 : c0 + nk],
                                         start=True, stop=True)
                        # fold edge weights into the gathered features
                        gtw = wpool.tile([128, nk, D], BF16, tag="w")
                        wsl = w_c[:, 0, :]
                        w_b = bass.AP(wsl.tensor, wsl.offset,
                                      [wsl.ap[0], [1, nk], [0, D]])
                        nc.vector.tensor_tensor(
                            gtw[:], gt[:, :, 0:D], w_b, op=mybir.AluOpType.mult)
                        # one-hot scatter matrices streamed from HBM (HWDGE,
                        # fp8 0/1 exact; replaces the DVE is_eq build)
                        st = spool.tile([128, nk, 128], F8, tag="s")
                        nc.sync.dma_start(
                            st[:],
                            bass.AP(s8_d, c0 * 128,
                                    [[total_chunks * 128, 128], [1, nk * 128]]))
                        for j, (cglob, g) in enumerate(chunks):
                            if g not in open_aggs:
                                agg = ps_agg.tile([128, D], F32, tag="agg")
                                open_aggs[g] = agg
                            nc.tensor.matmul(
                                open_aggs[g][:], st[:, j, :], gtw[:, j, :],
                                start=(cglob == first_chunk[g]),
                                stop=(cglob == last_chunk[g]))
                    for g in win["groups"]:
                        emit_epilogue(g, open_aggs.pop(g))
                assert not open_aggs

            def epilogue_to_table(g, agg, w_slice, dst_loc):
                """h = LR(agg) (layer>=2 path); write (h @ W_next) dup'd."""
                h_sb = epool.tile([128, D], BF16, tag="h")
                nc.scalar.activation(h_sb[:], agg[:],
                                     mybir.ActivationFunctionType.Lrelu,
                                     alpha=NEG_SLOPE)
                trp = ps_tr.tile([64, 128], BF16, tag="tr")
                nc.tensor.transpose(trp[:], h_sb[:], ident_t[:])
                trs = epool.tile([64, 128], BF16, tag="trs")
                nc.scalar.copy(trs[:], trp[:])
                tp = ps_w.tile([128, D], F32, tag="tw")
                nc.tensor.matmul(tp[:], trs[:], w_slice, start=True, stop=True)
                ts = epool.tile([128, 2 * D], BF16, tag="ts")
                nc.scalar.copy(ts[:, 0:D], tp[:])
                nc.scalar.copy(ts[:, D : 2 * D], tp[:])
                nc.sync.dma_start(
                    bass.AP(dst_loc, g * GROUP * 2 * D, [[2 * D, 128], [1, 2 * D]]),
                    ts[:])

            # ================= layer 1 =================
            def epi_l1(g, agg):
                # h1 = LR(agg @ W1); table2 = h1 @ W2
                a_sb = epool.tile([128, D], BF16, tag="h")
                nc.scalar.copy(a_sb[:], agg[:])
                trp = ps_tr.tile([64, 128], BF16, tag="tr")
                nc.tensor.transpose(trp[:], a_sb[:], ident_t[:])
                trs = epool.tile([64, 128], BF16, tag="trs")
                nc.scalar.copy(trs[:], trp[:])
                gp = ps_w.tile([64, 128], F32, tag="tw")
                nc.tensor.matmul(gp[:], ws_t[:, 0:D], trs[:], start=True, stop=True)
                h1T = epool.tile([64, 128], BF16, tag="h1t")
                nc.scalar.activation(h1T[:], gp[:],
                                     mybir.ActivationFunctionType.Lrelu,
                                     alpha=NEG_SLOPE)
                t2p = ps_w.tile([128, D], F32, tag="tw")
                nc.tensor.matmul(t2p[:], h1T[:], ws_t[:, D : 2 * D],
                                 start=True, stop=True)
                t2s = epool.tile([128, 2 * D], BF16, tag="ts")
                nc.scalar.copy(t2s[:, 0:D], t2p[:])
                nc.scalar.copy(t2s[:, D : 2 * D], t2p[:])
                nc.sync.dma_start(
                    bass.AP(g2_loc, g * GROUP * 2 * D, [[2 * D, 128], [1, 2 * D]]),
                    t2s[:])

            mck = [int(v) for v in
                   os.environ.get("KERNEL_MAXCK", "0,0,0").split(",")]
            mck = [v or MAX_CALL_CHUNKS for v in mck]

            apAB = table_view(xdup_d, n_nodes, offB_l1)
            do_layer(apAB, idx1_t, epi_l1, maxck=mck[0])

            nc.gpsimd.collective_compute(
                "AllGather", mybir.AluOpType.bypass,
                replica_groups=[list(range(NC))],
                ins=[g2_loc.ap().opt()], outs=[g2_full.ap().opt()])

            # ================= layer 2 =================
            apAB = table_view(g2_full, tab_rows, offB_l23)
            do_layer(apAB, idx23_t,
                     lambda g, agg: epilogue_to_table(
                         g, agg, ws_t[:, 2 * D : 3 * D], g3_loc),
                     maxck=mck[1])

            nc.gpsimd.collective_compute(
                "AllGather", mybir.AluOpType.bypass,
                replica_groups=[list(range(NC))],
                ins=[g3_loc.ap().opt()], outs=[g3_full.ap().opt()])

            # ================= layer 3 =================
            def epi_l3(g, agg):
                o_sb = epool.tile([128, D], F32, tag="o")
                nc.scalar.activation(o_sb[:], agg[:],
                                     mybir.ActivationFunctionType.Lrelu,
                                     alpha=NEG_SLOPE)
                rows = min(GROUP, nodes_per_core - g * GROUP)
                nc.sync.dma_start(
                    bass.AP(out_d, g * GROUP * D, [[D, rows], [1, D]]),
                    o_sb[0:rows, :])

            apAB = table_view(g3_full, tab_rows, offB_l23)
            do_layer(apAB, idx23_t, epi_l3, maxck=mck[2])

    nc.compile()
    return nc


def _run(x, edge_index, edge_w, W1, W2, W3):
    global LAST_EXEC_NS
    n_nodes = x.shape[0]
    nodes_per_core = n_nodes // NC
    src = np.asarray(edge_index[0], np.int64)
    dst = np.asarray(edge_index[1], np.int64)
    w = np.asarray(edge_w, np.float32)

    sched, per_core = _prep_edges(src, dst, w, n_nodes, nodes_per_core)

    nc = _build_nc(n_nodes, sched)

    xdup = np.concatenate([x, x], axis=1).astype(ml_dtypes.bfloat16)
    ident = np.eye(128, dtype=ml_dtypes.bfloat16)
    ws = np.concatenate([np.asarray(Wi, np.float32) for Wi in (W1, W2, W3)],
                        axis=0).astype(ml_dtypes.bfloat16)
    fp8 = mybir.dt.np(F8)
    tc = sched["total_chunks"]
    ci_ix = np.arange(tc)[:, None]
    ei_ix = np.arange(128)[None, :]

    in_maps = []
    for c in range(NC):
        pc = per_core[c]
        u8 = np.zeros((tc, 128, 128), np.uint8)
        u8[ci_ix, ei_ix, pc["dl"].astype(np.int64)] = 0x38  # 1.0 in e4m3
        s8 = np.ascontiguousarray(u8.transpose(1, 0, 2)).reshape(128, tc * 128)
        in_maps.append({
            "xdup": xdup,
            "idx1": _wrap_idx(pc["idx1"]),
            "idx23": _wrap_idx(pc["idx23"]),
            "wv": pc["w"].T.astype(ml_dtypes.bfloat16).copy(),
            "s8": s8.view(fp8),
            "ident": ident,
            "ws": ws,
        })

    trace = bool(int(os.environ.get("KERNEL_PROFILE", "0")))
    if trace:
        _install_profile_shim()
    res = run_bass_kernel_spmd(nc, in_maps, core_ids=list(range(NC)), trace=trace)
    LAST_EXEC_NS = res.exec_time_ns
    globals()["LAST_RESULTS"] = res.results
    out = np.concatenate([res.results[c]["out"] for c in range(NC)], axis=0)
    return out.astype(np.float32)


def kernel(x, edge_index, edge_w, W1, W2, W3):
    x = np.asarray(x, np.float32)
    assert x.shape == (50000, 64)
    return _run(x, np.asarray(edge_index), np.asarray(edge_w), W1, W2, W3)
